# revision 32
# baseline (speedup 1.0000x reference)
"""DynamiConv Trainium2 kernel.

out = gate * conv3x3(x, weight) + bias,  gate = conv3x3(sigmoid(x), dweight)

Strategy (per core, data-parallel over batch B=8 across 8 cores):
 - x96 [96, H, W+2] fp16 in SBUF: partition (ki*32+c) holds x[c, h+ki-1, w],
   built from one SWDGE cast-DMA (fp32->fp16) for the ki=1 block plus two
   partition-offset copies (DVE / GPSIMD) for ki=0 / ki=2. Guard columns
   (w=-1, w=256) and boundary rows are zeroed => conv zero padding.
 - s96 strips = sigmoid(x96 strip) on ACT (guards become 0.5, matching the
   reference which applies sigmoid to the zero-padded im2col).
 - Both convs are K=96 matmuls with the 3 horizontal taps (kj) accumulated in
   PSUM via rhs access-pattern shifts. 4 column-group lanes (tile_position)
   process 4 row-pairs concurrently: psumY[32j:32j+32] = conv rows, psumG
   gets the gate replicated over the 32 output channels.
 - out = psumY * psumG (DVE tensor_mul) + bias (tensor_scalar_add), DMA out.
"""

import os
import sys
import numpy as np
from contextlib import ExitStack

sys.path.insert(0, "/opt/trn_rl_repo")

C, O, KK = 32, 32, 3
W = 256
WG = W + 2          # guarded width (w = -1 .. 256)
TROWS = 8           # output rows per strip
LANES = 4           # column-group lanes; TROWS == 2 * LANES
NCORES = 8

_CACHE = {}


def _build_bass(H, evict_gate=True, reps=1, copyb_engine="vector", dataprep="swdge_cast",
                no_gate=False, mm_order="lane", skip_copies=False, dma_batch=4,
                out_batch=4, skip_out=False):
    import concourse.tile as tile
    from concourse import bacc, mybir

    f32 = mybir.dt.float32
    f16 = mybir.dt.float16

    nstrips = H // TROWS
    if nstrips % out_batch != 0:
        out_batch = 1
    nc = bacc.Bacc("TRN2")

    x_d = nc.dram_tensor("x", (C, H, W), f32, kind="ExternalInput")
    lm_d = nc.dram_tensor("lm", (KK, 96, O), f16, kind="ExternalInput")
    lg_d = nc.dram_tensor("lg", (KK, 96, O), f16, kind="ExternalInput")
    b_d = nc.dram_tensor("bias128", (128, 1), f32, kind="ExternalInput")
    out_d = nc.dram_tensor("out", (O, H, W), f32, kind="ExternalOutput")

    with tile.TileContext(nc) as tc, ExitStack() as ctx:
        singles = ctx.enter_context(tc.tile_pool(name="singles", bufs=1))
        stg_pool = ctx.enter_context(tc.tile_pool(name="stg", bufs=3))
        s_pool = ctx.enter_context(tc.tile_pool(name="s_pool", bufs=3))
        o_pool = ctx.enter_context(tc.tile_pool(name="o_pool", bufs=3))
        g_pool = ctx.enter_context(tc.tile_pool(name="g_pool", bufs=3))
        psum = ctx.enter_context(tc.tile_pool(name="psum", bufs=4, space="PSUM"))

        x96 = singles.tile([96, H, WG], f16)
        lm = singles.tile([96, KK, O], f16)
        lg = singles.tile([96, KK, O], f16)
        bias_sb = singles.tile([128, 1], f32)

        nc.sync.dma_start(out=lm, in_=lm_d[:, :, :].rearrange("k p m -> p k m"))
        nc.sync.dma_start(out=lg, in_=lg_d[:, :, :].rearrange("k p m -> p k m"))
        nc.sync.dma_start(out=bias_sb, in_=b_d[:, :])

        # zero pad guards: w guard columns, plus ki=0 row -1 and ki=2 row H
        nc.vector.memset(x96[:, :, 0:1], 0.0)
        nc.vector.memset(x96[:, :, WG - 1 : WG], 0.0)
        nc.vector.memset(x96[0:32, 0:1, :], 0.0)
        nc.vector.memset(x96[64:96, H - 1 : H, :], 0.0)

        def do_strip(t):
            r0 = t * TROWS
            # build replicated blocks for this strip
            if not skip_copies:
                a_lo = max(r0, 1)                 # ki=0 block: x[c, h-1]
                nc.vector.tensor_copy(
                    out=x96[0:32, a_lo : r0 + TROWS, :],
                    in_=x96[32:64, a_lo - 1 : r0 + TROWS - 1, :],
                )
                b_hi = min(r0 + TROWS, H - 1)     # ki=2 block: x[c, h+1]
                copyb = nc.gpsimd if copyb_engine == "gpsimd" else nc.vector
                copyb.tensor_copy(
                    out=x96[64:96, r0:b_hi, :],
                    in_=x96[32:64, r0 + 1 : b_hi + 1, :],
                )
            # sigmoid (fp16 in/out); guards -> 0.5 as required
            s_buf = s_pool.tile([96, TROWS, WG], f16)
            nc.scalar.activation(
                out=s_buf[:, :, :],
                in_=x96[:, r0 : r0 + TROWS, :],
                func=mybir.ActivationFunctionType.Sigmoid,
            )

            psumY = psum.tile([128, 2 * W], mybir.dt.float32)
            psumG = psum.tile([128, 2 * W], mybir.dt.float32)
            # "lane": each lane's accumulation group completes before the next
            # lane's start=True (CoreSim-conservative).  "phase": interleave
            # lanes per kj phase for max cross-column-group concurrency
            # (HW-validated correct; CoreSim's whole-bank zero-region model
            # rejects it, so only use after bypassing sim).
            def emit(psum_t, lhsT_t, rhs_fn):
                if mm_order == "lane":
                    order = [(j, kj) for j in range(LANES) for kj in range(KK)]
                else:
                    order = [(j, kj) for kj in range(KK) for j in range(LANES)]
                for j, kj in order:
                    nc.tensor.matmul(
                        psum_t[32 * j : 32 * j + 32, :],
                        lhsT=lhsT_t[:, kj, :],
                        rhs=rhs_fn(j, kj),
                        start=(kj == 0),
                        stop=(kj == KK - 1),
                        tile_position=(0, 32 * j),
                    )

            emit(psumY, lm, lambda j, kj: x96[:, r0 + 2 * j : r0 + 2 * j + 2, kj : kj + W])
            if not no_gate:
                emit(psumG, lg, lambda j, kj: s_buf[:, 2 * j : 2 * j + 2, kj : kj + W])

            if out_batch == 1:
                out_sb = o_pool.tile([128, 2 * W], f32)
                out_slot = out_sb
            else:
                if t % out_batch == 0:
                    do_strip.out_group = o_pool.tile([128, out_batch, 2 * W], f32)
                out_sb = do_strip.out_group
                out_slot = out_sb[:, t % out_batch, :]
            if no_gate:
                nc.vector.tensor_scalar_add(out_slot, psumY, bias_sb)
            elif evict_gate:
                g_sb = g_pool.tile([128, 2 * W], f16)
                nc.scalar.copy(out=g_sb, in_=psumG)
                nc.vector.tensor_mul(out=out_slot, in0=psumY, in1=g_sb)
                nc.vector.tensor_scalar_add(out_slot, out_slot, bias_sb)
            else:
                nc.vector.tensor_mul(out=out_slot, in0=psumY, in1=psumG)
                nc.vector.tensor_scalar_add(out_slot, out_slot, bias_sb)

            if skip_out:
                if t == nstrips - 1:  # keep out_d written so it isn't DCE'd
                    nc.sync.dma_start(
                        out=out_d[:, 0:2, :], in_=out_sb[0:32, 0 : 2 * W]
                    )
            elif out_batch == 1:
                for j in range(LANES):
                    rj = r0 + 2 * j
                    eng = nc.sync if j % 2 == 0 else nc.scalar
                    eng.dma_start(
                        out=out_d[:, rj : rj + 2, :],
                        in_=out_sb[32 * j : 32 * j + 32, :],
                    )
            elif t % out_batch == out_batch - 1:
                # one DMA per lane covering out_batch strips:
                # dram rows 8t'+2j+r for t' in group, viewed as [o, t', r, w]
                g0 = t - (out_batch - 1)
                ov = out_d[:, :, :].rearrange("o (tt r2) w -> o tt r2 w", r2=TROWS)
                for j in range(LANES):
                    eng = nc.sync if j % 2 == 0 else nc.scalar
                    eng.dma_start(
                        out=ov[:, g0 : g0 + out_batch, 2 * j : 2 * j + 2, :],
                        in_=out_sb[32 * j : 32 * j + 32, :, :],
                    )

        # software-pipelined emission: dma_in(i) ; full strip (i-1)
        def dma_in(i, nrows=TROWS):
            r0 = i * TROWS
            if dataprep == "swdge_cast":
                nc.gpsimd.dma_start(
                    out=x96[32:64, r0 : r0 + nrows, 1 : 1 + W],
                    in_=x_d[:, r0 : r0 + nrows, :],
                )
            elif dataprep == "stage_cast":  # HWDGE fp32 in, DVE cast-copy to fp16
                stg = stg_pool.tile([32, TROWS, W], f32)
                nc.sync.dma_start(out=stg, in_=x_d[:, r0 : r0 + TROWS, :])
                nc.vector.tensor_copy(
                    out=x96[32:64, r0 : r0 + TROWS, 1 : 1 + W], in_=stg
                )
            elif dataprep == "hwdge_probe":  # timing probe: plain fp32 DMA only
                stg = stg_pool.tile([32, TROWS, W], f32)
                nc.sync.dma_start(out=stg, in_=x_d[:, r0 : r0 + TROWS, :])
            elif dataprep == "none":  # timing probe: no input DMA at all
                pass

        def whole_image():
            for i in range(nstrips + 1):
                if i < nstrips:
                    if dma_batch == 1:
                        dma_in(i)
                    elif i % dma_batch == 0:
                        dma_in(i, nrows=min(dma_batch, nstrips - i) * TROWS)
                if i >= 1:
                    do_strip(i - 1)

        if reps == 1:
            whole_image()
        else:
            with tc.For_i(0, reps, 1):
                whole_image()

    nc.compile()
    return nc


def _pack_inputs(x_b, weight, dweight, bias):
    # lm[kj, ki*32+c, o] = weight[o, c, ki, kj]
    lm = np.ascontiguousarray(
        weight.transpose(3, 2, 1, 0).reshape(KK, KK * C, O).astype(np.float16)
    )
    # lg[kj, ki*32+c, o] = dweight[0, c, ki, kj] for all o
    lg = np.ascontiguousarray(
        np.broadcast_to(
            dweight.transpose(3, 2, 1, 0).reshape(KK, KK * C, 1), (KK, KK * C, O)
        ).astype(np.float16)
    )
    b128 = np.tile(np.asarray(bias, dtype=np.float32), 4).reshape(128, 1)
    return {
        "x": np.ascontiguousarray(x_b, dtype=np.float32),
        "lm": lm,
        "lg": lg,
        "bias128": np.ascontiguousarray(b128),
    }


def kernel(x, weight, dweight, bias):
    from concourse import bass_utils

    x = np.asarray(x)
    weight = np.asarray(weight)
    dweight = np.asarray(dweight)
    bias = np.asarray(bias)
    B, _, H, _ = x.shape

    key = ("nc", H)
    if key not in _CACHE:
        _CACHE[key] = _build_bass(H)
    nc = _CACHE[key]

    in_maps = [_pack_inputs(x[b], weight, dweight, bias) for b in range(B)]
    res = bass_utils.run_bass_kernel_spmd(
        nc,
        in_maps,
        core_ids=list(range(min(B, NCORES))),
        trace=bool(int(os.environ.get("DYNCONV_TRACE", "0"))),
    )
    out = np.stack([res.results[b]["out"] for b in range(B)], axis=0)
    kernel.last_results = res
    return out


# revision 37
# speedup vs baseline: 1.0240x; 1.0240x over previous
"""DynamiConv Trainium2 kernel.

out = gate * conv3x3(x, weight) + bias,  gate = conv3x3(sigmoid(x), dweight)

Strategy (per core, data-parallel over batch B=8 across 8 cores):
 - x96 [96, H, W+2] fp16 in SBUF: partition (ki*32+c) holds x[c, h+ki-1, w],
   built from one SWDGE cast-DMA (fp32->fp16) for the ki=1 block plus two
   partition-offset copies (DVE / GPSIMD) for ki=0 / ki=2. Guard columns
   (w=-1, w=256) and boundary rows are zeroed => conv zero padding.
 - s96 strips = sigmoid(x96 strip) on ACT (guards become 0.5, matching the
   reference which applies sigmoid to the zero-padded im2col).
 - Both convs are K=96 matmuls with the 3 horizontal taps (kj) accumulated in
   PSUM via rhs access-pattern shifts. 4 column-group lanes (tile_position)
   process 4 row-pairs concurrently: psumY[32j:32j+32] = conv rows, psumG
   gets the gate replicated over the 32 output channels.
 - out = psumY * psumG (DVE tensor_mul) + bias (tensor_scalar_add), DMA out.
"""

import os
import sys
import numpy as np
from contextlib import ExitStack

sys.path.insert(0, "/opt/trn_rl_repo")

C, O, KK = 32, 32, 3
W = 256
WG = W + 2          # guarded width (w = -1 .. 256)
TROWS = 8           # output rows per strip
LANES = 4           # column-group lanes; TROWS == 2 * LANES
NCORES = 8

_CACHE = {}


def _build_bass(H, evict_gate=True, reps=1, copyb_engine="vector", dataprep="swdge_cast",
                no_gate=False, mm_order="lane", skip_copies=False, dma_batch=4,
                out_batch=4, skip_out=False, prep_fuse=1):
    import concourse.tile as tile
    from concourse import bacc, mybir

    f32 = mybir.dt.float32
    f16 = mybir.dt.float16

    nstrips = H // TROWS
    if nstrips % out_batch != 0:
        out_batch = 1
    nc = bacc.Bacc("TRN2")

    x_d = nc.dram_tensor("x", (C, H, W), f32, kind="ExternalInput")
    lm_d = nc.dram_tensor("lm", (KK, 96, O), f16, kind="ExternalInput")
    lg_d = nc.dram_tensor("lg", (KK, 96, O), f16, kind="ExternalInput")
    b_d = nc.dram_tensor("bias128", (128, 1), f32, kind="ExternalInput")
    out_d = nc.dram_tensor("out", (O, H, W), f32, kind="ExternalOutput")

    with tile.TileContext(nc) as tc, ExitStack() as ctx:
        singles = ctx.enter_context(tc.tile_pool(name="singles", bufs=1))
        stg_pool = ctx.enter_context(tc.tile_pool(name="stg", bufs=3))
        s_pool = ctx.enter_context(tc.tile_pool(name="s_pool", bufs=3))
        o_pool = ctx.enter_context(tc.tile_pool(name="o_pool", bufs=3))
        g_pool = ctx.enter_context(tc.tile_pool(name="g_pool", bufs=3))
        psum = ctx.enter_context(tc.tile_pool(name="psum", bufs=4, space="PSUM"))

        x96 = singles.tile([96, H, WG], f16)
        lm = singles.tile([96, KK, O], f16)
        lg = singles.tile([96, KK, O], f16)
        bias_sb = singles.tile([128, 1], f32)

        nc.sync.dma_start(out=lm, in_=lm_d[:, :, :].rearrange("k p m -> p k m"))
        nc.sync.dma_start(out=lg, in_=lg_d[:, :, :].rearrange("k p m -> p k m"))
        nc.sync.dma_start(out=bias_sb, in_=b_d[:, :])

        # zero pad guards: w guard columns, plus ki=0 row -1 and ki=2 row H
        nc.vector.memset(x96[:, :, 0:1], 0.0)
        nc.vector.memset(x96[:, :, WG - 1 : WG], 0.0)
        nc.vector.memset(x96[0:32, 0:1, :], 0.0)
        nc.vector.memset(x96[64:96, H - 1 : H, :], 0.0)

        def do_prep(t, nrows):
            r0 = t * TROWS
            if not skip_copies:
                a_lo = max(r0, 1)                 # ki=0 block: x[c, h-1]
                nc.vector.tensor_copy(
                    out=x96[0:32, a_lo : r0 + nrows, :],
                    in_=x96[32:64, a_lo - 1 : r0 + nrows - 1, :],
                )
                b_hi = min(r0 + nrows, H - 1)     # ki=2 block: x[c, h+1]
                copyb = nc.gpsimd if copyb_engine == "gpsimd" else nc.vector
                copyb.tensor_copy(
                    out=x96[64:96, r0:b_hi, :],
                    in_=x96[32:64, r0 + 1 : b_hi + 1, :],
                )
            # sigmoid (fp16 in/out); guards -> 0.5 as required
            s_buf = s_pool.tile([96, prep_fuse * TROWS, WG], f16)
            nc.scalar.activation(
                out=s_buf[:, :nrows, :],
                in_=x96[:, r0 : r0 + nrows, :],
                func=mybir.ActivationFunctionType.Sigmoid,
            )
            do_prep.s_buf = s_buf

        def do_strip(t):
            r0 = t * TROWS
            if t % prep_fuse == 0:
                do_prep(t, min(prep_fuse, nstrips - t) * TROWS)
            s_buf = do_prep.s_buf
            s_off = (t % prep_fuse) * TROWS

            psumY = psum.tile([128, 2 * W], mybir.dt.float32)
            psumG = psum.tile([128, 2 * W], mybir.dt.float32)
            # "lane": each lane's accumulation group completes before the next
            # lane's start=True (CoreSim-conservative).  "phase": interleave
            # lanes per kj phase for max cross-column-group concurrency
            # (HW-validated correct; CoreSim's whole-bank zero-region model
            # rejects it, so only use after bypassing sim).
            def emit(psum_t, lhsT_t, rhs_fn):
                if mm_order == "lane":
                    order = [(j, kj) for j in range(LANES) for kj in range(KK)]
                else:
                    order = [(j, kj) for kj in range(KK) for j in range(LANES)]
                for j, kj in order:
                    nc.tensor.matmul(
                        psum_t[32 * j : 32 * j + 32, :],
                        lhsT=lhsT_t[:, kj, :],
                        rhs=rhs_fn(j, kj),
                        start=(kj == 0),
                        stop=(kj == KK - 1),
                        tile_position=(0, 32 * j),
                    )

            emit(psumY, lm, lambda j, kj: x96[:, r0 + 2 * j : r0 + 2 * j + 2, kj : kj + W])
            if not no_gate:
                emit(psumG, lg,
                     lambda j, kj: s_buf[:, s_off + 2 * j : s_off + 2 * j + 2, kj : kj + W])

            if out_batch == 1:
                out_sb = o_pool.tile([128, 2 * W], f32)
                out_slot = out_sb
            else:
                if t % out_batch == 0:
                    do_strip.out_group = o_pool.tile([128, out_batch, 2 * W], f32)
                out_sb = do_strip.out_group
                out_slot = out_sb[:, t % out_batch, :]
            if no_gate:
                nc.vector.tensor_scalar_add(out_slot, psumY, bias_sb)
            elif evict_gate:
                g_sb = g_pool.tile([128, 2 * W], f16)
                nc.scalar.copy(out=g_sb, in_=psumG)
                nc.vector.tensor_mul(out=out_slot, in0=psumY, in1=g_sb)
                nc.vector.tensor_scalar_add(out_slot, out_slot, bias_sb)
            else:
                nc.vector.tensor_mul(out=out_slot, in0=psumY, in1=psumG)
                nc.vector.tensor_scalar_add(out_slot, out_slot, bias_sb)

            if skip_out:
                if t == nstrips - 1:  # keep out_d written so it isn't DCE'd
                    nc.sync.dma_start(
                        out=out_d[:, 0:2, :], in_=out_sb[0:32, 0 : 2 * W]
                    )
            elif out_batch == 1:
                for j in range(LANES):
                    rj = r0 + 2 * j
                    eng = nc.sync if j % 2 == 0 else nc.scalar
                    eng.dma_start(
                        out=out_d[:, rj : rj + 2, :],
                        in_=out_sb[32 * j : 32 * j + 32, :],
                    )
            elif t % out_batch == out_batch - 1:
                # one DMA per lane covering out_batch strips:
                # dram rows 8t'+2j+r for t' in group, viewed as [o, t', r, w]
                g0 = t - (out_batch - 1)
                ov = out_d[:, :, :].rearrange("o (tt r2) w -> o tt r2 w", r2=TROWS)
                for j in range(LANES):
                    eng = nc.sync if j % 2 == 0 else nc.scalar
                    eng.dma_start(
                        out=ov[:, g0 : g0 + out_batch, 2 * j : 2 * j + 2, :],
                        in_=out_sb[32 * j : 32 * j + 32, :, :],
                    )

        # software-pipelined emission: dma_in(i) ; full strip (i-1)
        def dma_in(i, nrows=TROWS):
            r0 = i * TROWS
            if dataprep == "swdge_cast":
                nc.gpsimd.dma_start(
                    out=x96[32:64, r0 : r0 + nrows, 1 : 1 + W],
                    in_=x_d[:, r0 : r0 + nrows, :],
                )
            elif dataprep == "stage_cast":  # HWDGE fp32 in, DVE cast-copy to fp16
                stg = stg_pool.tile([32, TROWS, W], f32)
                nc.sync.dma_start(out=stg, in_=x_d[:, r0 : r0 + TROWS, :])
                nc.vector.tensor_copy(
                    out=x96[32:64, r0 : r0 + TROWS, 1 : 1 + W], in_=stg
                )
            elif dataprep == "hwdge_probe":  # timing probe: plain fp32 DMA only
                stg = stg_pool.tile([32, TROWS, W], f32)
                nc.sync.dma_start(out=stg, in_=x_d[:, r0 : r0 + TROWS, :])
            elif dataprep == "none":  # timing probe: no input DMA at all
                pass

        def whole_image():
            for i in range(nstrips + 1):
                if i < nstrips:
                    if dma_batch == 1:
                        dma_in(i)
                    elif i % dma_batch == 0:
                        dma_in(i, nrows=min(dma_batch, nstrips - i) * TROWS)
                if i >= 1:
                    do_strip(i - 1)

        if reps == 1:
            whole_image()
        else:
            with tc.For_i(0, reps, 1):
                whole_image()

    nc.compile()
    return nc


def _pack_inputs(x_b, weight, dweight, bias):
    # lm[kj, ki*32+c, o] = weight[o, c, ki, kj]
    lm = np.ascontiguousarray(
        weight.transpose(3, 2, 1, 0).reshape(KK, KK * C, O).astype(np.float16)
    )
    # lg[kj, ki*32+c, o] = dweight[0, c, ki, kj] for all o
    lg = np.ascontiguousarray(
        np.broadcast_to(
            dweight.transpose(3, 2, 1, 0).reshape(KK, KK * C, 1), (KK, KK * C, O)
        ).astype(np.float16)
    )
    b128 = np.tile(np.asarray(bias, dtype=np.float32), 4).reshape(128, 1)
    return {
        "x": np.ascontiguousarray(x_b, dtype=np.float32),
        "lm": lm,
        "lg": lg,
        "bias128": np.ascontiguousarray(b128),
    }


def kernel(x, weight, dweight, bias):
    from concourse import bass_utils

    x = np.asarray(x)
    weight = np.asarray(weight)
    dweight = np.asarray(dweight)
    bias = np.asarray(bias)
    B, _, H, _ = x.shape

    key = ("nc", H)
    if key not in _CACHE:
        _CACHE[key] = _build_bass(H)
    nc = _CACHE[key]

    in_maps = [_pack_inputs(x[b], weight, dweight, bias) for b in range(B)]
    res = bass_utils.run_bass_kernel_spmd(
        nc,
        in_maps,
        core_ids=list(range(min(B, NCORES))),
        trace=bool(int(os.environ.get("DYNCONV_TRACE", "0"))),
    )
    out = np.stack([res.results[b]["out"] for b in range(B)], axis=0)
    kernel.last_results = res
    return out
